# revision 23
# baseline (speedup 1.0000x reference)
"""Block-diagonal grouped GEMM (GroupLinear) on 8 TRN2 NeuronCores.

Problem: x [8, 2048, 4096] f32, W [4096, 4096] f32 where only the 64
diagonal 64x64 blocks of W are used:
    y[b,s, g*64+o] = sum_i x[b,s, g*64+i] * W[g*64+o, g*64+i]

Strategy:
  - Channel-parallel across cores: core m owns 4 of the 32 128-channel
    strips (8 groups) for ALL batches, so it loads only 128KB of
    weights instead of the full 1MB -> less HBM traffic per core.
    Work per core: 32 virtual strips (4 local strips x 8 batches),
    each [128ch x 2048 tok].
  - All device I/O in bf16 (rel-err budget 2e-2; bf16 costs ~3e-3),
    halving HBM traffic vs f32: 16MB x + 16MB y + 128KB W per core
    against the ~358 GB/s per-core HBM limit -> ~92us floor.
  - Host pre-packs x strip-major so the PE contraction dim (channel)
    is on partitions and each load DMA is contiguous per partition.
  - Two 64-ch groups pack into one 128-wide block-diagonal weight
    strip [128i, 128o]; 8 consecutive virtual strips share weights.
  - Per 4-strip chunk (2MB load DMA at ~97% DMA efficiency): per
    strip 4 matmuls into 2-bank PSUM tiles, half-strip [128,1024]
    PSUM->SBUF copy+cast alternating Vector/Scalar, then one 2MB
    store.  Loads ride the Sync HWDGE ring, stores the Scalar ring.
  - Host un-permutes ys and upcasts to f32.
"""

import numpy as np
from ml_dtypes import bfloat16

import concourse.bacc as bacc
import concourse.mybir as mybir
from concourse.tile import TileContext
from concourse.bass_utils import run_bass_kernel_spmd

B, S, C = 8, 2048, 4096
G, GS = 64, 64            # groups, group size (=in_scale=out_scale)
NSTRIP = C // 128         # 32 strips of 128 channels (2 groups each)
LSTRIP = NSTRIP // 8      # 4 local strips per core
NV = LSTRIP * B           # 32 virtual strips per core (local strip x batch)
CHUNK = 4                 # virtual strips per load/store DMA (2MB)
NCHUNK = NV // CHUNK
TOK = 512                 # PSUM bank limit: 512 f32 per partition
FP32 = mybir.dt.float32
BF16 = mybir.dt.bfloat16


def _build_program():
    nc = bacc.Bacc()
    xs = nc.declare_dram_parameter("xs", [128, NV * S], BF16, isOutput=False)
    wb = nc.declare_dram_parameter("wb", [128, LSTRIP * 128], BF16, isOutput=False)
    ys = nc.declare_dram_parameter("ys", [128, NV * S], BF16, isOutput=True)

    with TileContext(nc) as tc:
        with (
            tc.tile_pool(name="wpool", bufs=1) as wpool,
            tc.tile_pool(name="xpool", bufs=3) as xpool,
            tc.tile_pool(name="opool", bufs=4) as opool,
            tc.tile_pool(name="ppool", bufs=4, space="PSUM") as ppool,
        ):
            # 128KB of weights: one DMA on the (otherwise idle at
            # start) scalar ring so the first x strip isn't queued
            # behind it on sync.
            w_sb = wpool.tile([128, LSTRIP * 128], BF16)
            nc.scalar.dma_start(out=w_sb[:], in_=wb[:])
            # Mixed chunk plan: 1MB chunks early (fast pool recycling
            # while compute is still cold), 2MB after.
            plan = [2, 2] + [CHUNK] * ((NV - 4) // CHUNK)
            v0 = 0
            for cc, csz in enumerate(plan):
                x_t = xpool.tile([128, csz * S], BF16)
                lo = v0 * S
                if cc == 0:
                    # Pipeline fill: stores don't exist yet, so use
                    # BOTH rings for the first chunk (strip-granular
                    # so the first matmul starts ~0.5MB in).
                    for k in range(csz):
                        eng = nc.sync if k % 2 == 0 else nc.scalar
                        eng.dma_start(
                            out=x_t[:, k * S:(k + 1) * S],
                            in_=xs[:, lo + k * S:lo + (k + 1) * S],
                        )
                else:
                    nc.sync.dma_start(out=x_t[:], in_=xs[:, lo:lo + csz * S])
                o_t = opool.tile([128, csz * S], BF16)
                for k in range(csz):
                    v = v0 + k
                    ls = v // B          # local strip = shared weights
                    for h in range(2):
                        ps = ppool.tile([128, 2 * TOK], FP32)  # 2 PSUM banks
                        for q in range(2):
                            col = (2 * h + q) * TOK
                            nc.tensor.matmul(
                                out=ps[:, q * TOK:(q + 1) * TOK],
                                lhsT=w_sb[:, ls * 128:(ls + 1) * 128],
                                rhs=x_t[:, k * S + col:k * S + col + TOK],
                                start=True,
                                stop=True,
                            )
                        # Half-strip copy+cast, engines alternate: finer
                        # PSUM recycling so PE never waits a full strip.
                        dst = o_t[:, k * S + h * 2 * TOK:k * S + (h + 1) * 2 * TOK]
                        if (2 * v + h) % 2 == 0:
                            nc.vector.tensor_copy(out=dst, in_=ps[:])
                        else:
                            nc.scalar.copy(out=dst, in_=ps[:])
                if cc == len(plan) - 1:
                    # Drain: strip-granular stores alternating rings so
                    # the final transfer tail is ~0.5MB, not 2MB.
                    for k in range(csz):
                        eng = nc.scalar if k % 2 == 0 else nc.sync
                        eng.dma_start(
                            out=ys[:, lo + k * S:lo + (k + 1) * S],
                            in_=o_t[:, k * S:(k + 1) * S],
                        )
                else:
                    nc.scalar.dma_start(
                        out=ys[:, lo:lo + csz * S], in_=o_t[:]
                    )
                v0 += csz
    nc.finalize()
    return nc


def _prep_in_maps(x, W):
    # Diagonal blocks: Wdiag[g][o, i] = W[g*64+o, g*64+i]
    Wr = W.reshape(G, GS, G, GS)
    g = np.arange(G)
    WdT = Wr[g, :, g, :].transpose(0, 2, 1)          # [g, i, o]
    wb = np.zeros((128, NSTRIP, 128), dtype=np.float32)
    for c in range(NSTRIP):
        wb[0:64, c, 0:64] = WdT[2 * c]
        wb[64:128, c, 64:128] = WdT[2 * c + 1]
    wb = wb.astype(bfloat16)
    # x [B, S, NSTRIP, 128] -> per core m: strips 4m..4m+3, laid out
    # [128p, (local strip, batch), S] so consecutive virtual strips
    # share weights.
    xv = np.ascontiguousarray(
        x.reshape(B, S, NSTRIP, 128).transpose(3, 2, 0, 1)
    )  # [128, NSTRIP, B, S]
    maps = []
    for m in range(B):
        xs = xv[:, LSTRIP * m:LSTRIP * (m + 1)].reshape(128, NV * S)
        wm = np.ascontiguousarray(
            wb[:, LSTRIP * m:LSTRIP * (m + 1)].reshape(128, LSTRIP * 128)
        )
        maps.append({"xs": np.ascontiguousarray(xs).astype(bfloat16), "wb": wm})
    return maps


def run(x, W, trace=False, **kw):
    x = np.asarray(x, dtype=np.float32)
    W = np.asarray(W, dtype=np.float32)
    nc = _build_program()
    in_maps = _prep_in_maps(x, W)
    res = run_bass_kernel_spmd(nc, in_maps, list(range(B)), trace=trace, **kw)
    # ys_m [128, (local strip, batch), S] -> y [B, S, C]
    yv = np.empty((128, NSTRIP, B, S), dtype=np.float32)
    for m in range(B):
        yv[:, LSTRIP * m:LSTRIP * (m + 1)] = (
            res.results[m]["ys"].reshape(128, LSTRIP, B, S).astype(np.float32)
        )
    y = np.ascontiguousarray(yv.transpose(2, 3, 1, 0).reshape(B, S, C))
    return y, res


def kernel(x, W):
    y, _ = run(x, W, trace=False)
    return y


# revision 24
# speedup vs baseline: 1.1437x; 1.1437x over previous
"""Block-diagonal grouped GEMM (GroupLinear) on 8 TRN2 NeuronCores.

Problem: x [8, 2048, 4096] f32, W [4096, 4096] f32 where only the 64
diagonal 64x64 blocks of W are used:
    y[b,s, g*64+o] = sum_i x[b,s, g*64+i] * W[g*64+o, g*64+i]

Strategy:
  - Channel-parallel across cores: core m owns 4 of the 32 128-channel
    strips (8 groups) for ALL batches, so it loads only 128KB of
    weights instead of the full 1MB -> less HBM traffic per core.
    Work per core: 32 virtual strips (4 local strips x 8 batches),
    each [128ch x 2048 tok].
  - All device I/O in bf16 (rel-err budget 2e-2; bf16 costs ~3e-3),
    halving HBM traffic vs f32: 16MB x + 16MB y + 128KB W per core
    against the ~358 GB/s per-core HBM limit -> ~92us floor.
  - Host pre-packs x strip-major so the PE contraction dim (channel)
    is on partitions and each load DMA is contiguous per partition.
  - Two 64-ch groups pack into one 128-wide block-diagonal weight
    strip [128i, 128o]; 8 consecutive virtual strips share weights.
  - Per 4-strip chunk (2MB load DMA at ~97% DMA efficiency): per
    strip 4 matmuls into 2-bank PSUM tiles, half-strip [128,1024]
    PSUM->SBUF copy+cast alternating Vector/Scalar, then one 2MB
    store.  Loads ride the Sync HWDGE ring, stores the Scalar ring.
  - Host un-permutes ys and upcasts to f32.
"""

import numpy as np
from ml_dtypes import bfloat16

import concourse.bacc as bacc
import concourse.mybir as mybir
from concourse.tile import TileContext
from concourse.bass_utils import run_bass_kernel_spmd

B, S, C = 8, 2048, 4096
G, GS = 64, 64            # groups, group size (=in_scale=out_scale)
NSTRIP = C // 128         # 32 strips of 128 channels (2 groups each)
LSTRIP = NSTRIP // 8      # 4 local strips per core
NV = LSTRIP * B           # 32 virtual strips per core (local strip x batch)
CHUNK = 4                 # virtual strips per load/store DMA (2MB)
NCHUNK = NV // CHUNK
TOK = 512                 # PSUM bank limit: 512 f32 per partition
FP32 = mybir.dt.float32
BF16 = mybir.dt.bfloat16


def _build_program():
    nc = bacc.Bacc()
    xs = nc.declare_dram_parameter("xs", [128, NV * S], BF16, isOutput=False)
    wb = nc.declare_dram_parameter("wb", [128, LSTRIP * 128], BF16, isOutput=False)
    ys = nc.declare_dram_parameter("ys", [128, NV * S], BF16, isOutput=True)

    with TileContext(nc) as tc:
        with (
            tc.tile_pool(name="wpool", bufs=1) as wpool,
            tc.tile_pool(name="xpool", bufs=3) as xpool,
            tc.tile_pool(name="opool", bufs=4) as opool,
            tc.tile_pool(name="ppool", bufs=4, space="PSUM") as ppool,
        ):
            # 128KB of weights: one DMA on the (otherwise idle at
            # start) scalar ring so the first x strip isn't queued
            # behind it on sync.
            w_sb = wpool.tile([128, LSTRIP * 128], BF16)
            nc.scalar.dma_start(out=w_sb[:], in_=wb[:])
            # Mixed chunk plan: 1MB chunks early (fast pool recycling
            # while compute is still cold), 2MB after.
            plan = [2, 2] + [CHUNK] * ((NV - 4) // CHUNK)
            v0 = 0
            for cc, csz in enumerate(plan):
                x_t = xpool.tile([128, csz * S], BF16)
                lo = v0 * S
                if cc <= 2:
                    # Pipeline fill: stores don't flow until ~13us, so
                    # use BOTH rings for the first chunks (strip- or
                    # half-granular) to reach full bus rate sooner.
                    npc = 2 if csz == 2 else csz // 2
                    w_ = csz * S // npc
                    for k in range(npc):
                        eng = nc.sync if k % 2 == 0 else nc.scalar
                        eng.dma_start(
                            out=x_t[:, k * w_:(k + 1) * w_],
                            in_=xs[:, lo + k * w_:lo + (k + 1) * w_],
                        )
                else:
                    nc.sync.dma_start(out=x_t[:], in_=xs[:, lo:lo + csz * S])
                o_t = opool.tile([128, csz * S], BF16)
                for k in range(csz):
                    v = v0 + k
                    ls = v // B          # local strip = shared weights
                    for h in range(2):
                        ps = ppool.tile([128, 2 * TOK], FP32)  # 2 PSUM banks
                        for q in range(2):
                            col = (2 * h + q) * TOK
                            nc.tensor.matmul(
                                out=ps[:, q * TOK:(q + 1) * TOK],
                                lhsT=w_sb[:, ls * 128:(ls + 1) * 128],
                                rhs=x_t[:, k * S + col:k * S + col + TOK],
                                start=True,
                                stop=True,
                            )
                        # Half-strip copy+cast, engines alternate: finer
                        # PSUM recycling so PE never waits a full strip.
                        dst = o_t[:, k * S + h * 2 * TOK:k * S + (h + 1) * 2 * TOK]
                        if (2 * v + h) % 2 == 0:
                            nc.vector.tensor_copy(out=dst, in_=ps[:])
                        else:
                            nc.scalar.copy(out=dst, in_=ps[:])
                if cc == len(plan) - 1:
                    # Drain: strip-granular stores alternating rings so
                    # the final transfer tail is ~0.5MB, not 2MB.
                    for k in range(csz):
                        eng = nc.scalar if k % 2 == 0 else nc.sync
                        eng.dma_start(
                            out=ys[:, lo + k * S:lo + (k + 1) * S],
                            in_=o_t[:, k * S:(k + 1) * S],
                        )
                else:
                    nc.scalar.dma_start(
                        out=ys[:, lo:lo + csz * S], in_=o_t[:]
                    )
                v0 += csz
    nc.finalize()
    return nc


def _prep_in_maps(x, W):
    # Diagonal blocks: Wdiag[g][o, i] = W[g*64+o, g*64+i]
    Wr = W.reshape(G, GS, G, GS)
    g = np.arange(G)
    WdT = Wr[g, :, g, :].transpose(0, 2, 1)          # [g, i, o]
    wb = np.zeros((128, NSTRIP, 128), dtype=np.float32)
    for c in range(NSTRIP):
        wb[0:64, c, 0:64] = WdT[2 * c]
        wb[64:128, c, 64:128] = WdT[2 * c + 1]
    wb = wb.astype(bfloat16)
    # x [B, S, NSTRIP, 128] -> per core m: strips 4m..4m+3, laid out
    # [128p, (local strip, batch), S] so consecutive virtual strips
    # share weights.
    xv = np.ascontiguousarray(
        x.reshape(B, S, NSTRIP, 128).transpose(3, 2, 0, 1)
    )  # [128, NSTRIP, B, S]
    maps = []
    for m in range(B):
        xs = xv[:, LSTRIP * m:LSTRIP * (m + 1)].reshape(128, NV * S)
        wm = np.ascontiguousarray(
            wb[:, LSTRIP * m:LSTRIP * (m + 1)].reshape(128, LSTRIP * 128)
        )
        maps.append({"xs": np.ascontiguousarray(xs).astype(bfloat16), "wb": wm})
    return maps


def run(x, W, trace=False, **kw):
    x = np.asarray(x, dtype=np.float32)
    W = np.asarray(W, dtype=np.float32)
    nc = _build_program()
    in_maps = _prep_in_maps(x, W)
    res = run_bass_kernel_spmd(nc, in_maps, list(range(B)), trace=trace, **kw)
    # ys_m [128, (local strip, batch), S] -> y [B, S, C]
    yv = np.empty((128, NSTRIP, B, S), dtype=np.float32)
    for m in range(B):
        yv[:, LSTRIP * m:LSTRIP * (m + 1)] = (
            res.results[m]["ys"].reshape(128, LSTRIP, B, S).astype(np.float32)
        )
    y = np.ascontiguousarray(yv.transpose(2, 3, 1, 0).reshape(B, S, C))
    return y, res


def kernel(x, W):
    y, _ = run(x, W, trace=False)
    return y
